# revision 1
# baseline (speedup 1.0000x reference)
"""Trainium2 Bass kernel for DynamicGNN (3-layer RGCN-style message passing).

Strategy: shard destination nodes (and their incoming edges) across the 8
NeuronCores. Each core owns N/8 nodes = 3*N/8 (node,relation) segments.
Messages are gathered per-edge from a replicated node-feature table in DRAM
via dma_gather, segment-reduced with selection-matrix matmuls on the
TensorEngine, then transformed per-relation and layer-normed. Node tables
are rebuilt each layer with an AllGather collective.
"""
import math
import sys

import numpy as np

sys.path.insert(0, "/opt/trn_rl_repo")

NCORES = 8
HALF = 32768          # int16 index limit for dma_gather -> split table in 2
WINSEG = 128          # segments per psum window
BATCH_TILES = 8       # 128-edge tiles per dma_gather call (desc-ring limit ~1024)
LN_EPS = 1e-5
NUM_REL = 3

# debug knobs (bisect device hangs)
DBG_NO_COLLECTIVE = False
DBG_NO_ACCUM = False
DBG_NO_GATHER = False
DBG_SKIP = set()     # subset of {'seg','den','post','ln'}
DBG_LAYERS = None    # limit layer count


def _ceil(a, b):
    return (a + b - 1) // b


def _preprocess(x, edge_index, edge_type):
    """Host-side: shard edges by dst owner, sort by segment, build windows,
    A/B phase slot streams, gather indices and per-tile segment metadata."""
    N = x.shape[0]
    E = edge_index.shape[1]
    n_own = N // NCORES
    seg_per_core = n_own * NUM_REL
    nwin = _ceil(seg_per_core, WINSEG)

    src = edge_index[0].astype(np.int64)
    dst = edge_index[1].astype(np.int64)
    et = edge_type.astype(np.int64)

    # global per-(node,rel) counts -> mean denominators
    segg = dst * NUM_REL + et
    counts = np.bincount(segg, minlength=N * NUM_REL).astype(np.float32)
    denom_inv = 1.0 / np.maximum(counts, 1.0)          # [N*R]

    owner = dst // n_own
    cores = []
    # per (phase, window): edge counts per core
    cntA = np.zeros((NCORES, nwin), dtype=np.int64)
    cntB = np.zeros((NCORES, nwin), dtype=np.int64)
    per_core = []
    for c in range(NCORES):
        m = owner == c
        s_c = src[m]
        seg_c = (dst[m] - c * n_own) * NUM_REL + et[m]
        order = np.argsort(seg_c, kind="stable")
        s_c = s_c[order]
        seg_c = seg_c[order]
        w_c = seg_c // WINSEG
        isA = s_c < HALF
        cntA[c] = np.bincount(w_c[isA], minlength=nwin)
        cntB[c] = np.bincount(w_c[~isA], minlength=nwin)
        per_core.append((s_c, seg_c, w_c, isA))

    # compile-time tile structure: tiles per (phase, window) = max over cores
    tilesA = _ceil(np.maximum(cntA.max(axis=0), 0), 128)   # [nwin]
    tilesB = _ceil(np.maximum(cntB.max(axis=0), 0), 128)
    TA, TB = int(tilesA.sum()), int(tilesB.sum())
    slotsA, slotsB = TA * 128, TB * 128
    tbaseA = np.concatenate([[0], np.cumsum(tilesA)[:-1]])
    tbaseB = np.concatenate([[0], np.cumsum(tilesB)[:-1]])

    for c in range(NCORES):
        s_c, seg_c, w_c, isA = per_core[c]
        idxA = np.zeros(slotsA, dtype=np.int16)
        idxB = np.zeros(slotsB, dtype=np.int16)
        relA = np.full(slotsA, -1.0, dtype=np.float32)
        relB = np.full(slotsB, -1.0, dtype=np.float32)
        for (mask, idxv, relv, tbase, cnt, off) in (
            (isA, idxA, relA, tbaseA, cntA[c], 0),
            (~isA, idxB, relB, tbaseB, cntB[c], HALF),
        ):
            s_p = s_c[mask] - off
            seg_p = seg_c[mask]
            w_p = w_c[mask]
            gc = np.bincount(w_p, minlength=nwin)
            starts = np.concatenate([[0], np.cumsum(gc)[:-1]])
            pos = np.arange(len(w_p)) - starts[w_p]
            slot = tbase[w_p] * 128 + pos
            idxv[slot] = s_p.astype(np.int16)
            relv[slot] = (seg_p - w_p * WINSEG).astype(np.float32)
        cores.append((idxA, idxB, relA, relB))

    # partition packing: windows [0, wsplit) live on partitions 0:64 of S_T,
    # the rest on 64:128. wsplit is a multiple of 12 windows so 512-node
    # matmul chunks never straddle halves.
    segp_pad = _ceil(nwin * WINSEG, 3 * 512) * (3 * 512)
    meta = dict(
        N=N, E=E, n_own=n_own, seg_per_core=seg_per_core, nwin=nwin,
        tilesA=tilesA, tilesB=tilesB, TA=TA, TB=TB, segp_pad=segp_pad,
    )
    return meta, cores, denom_inv


def _pack_idx(flat):
    """[S] int16 -> [128, S/16] with the 16-wrap block replicated across the
    8 gpsimd cores."""
    blk = flat.reshape(-1, 16).T        # [16, S/16]
    return np.tile(blk, (8, 1)).copy()


def _pack_rel(flat):
    """[S] f32 -> [128, S/128]: slot i -> partition i%128, tile i//128."""
    return flat.reshape(-1, 128).T.copy()


def _build_program(meta, dt_np=np.float32):
    import concourse.bacc as bacc
    import concourse.bass as bass
    import concourse.mybir as mybir
    import concourse.tile as tile
    from concourse.masks import make_identity

    dt = mybir.dt
    f32 = dt.float32
    N = meta["N"]
    n_own = meta["n_own"]
    nwin = meta["nwin"]
    tilesA, tilesB = meta["tilesA"], meta["tilesB"]
    TA, TB = meta["TA"], meta["TB"]
    segp_pad = meta["segp_pad"]
    O = 64
    L = 3
    IN_DIM = 5
    rowsA = HALF if N > HALF else N     # rows in table half A

    nc = bacc.Bacc("TRN2", target_bir_lowering=False, debug=False,
                   enable_asserts=False, num_devices=NCORES)

    # ---- I/O ----
    def din(name, shape, d=f32):
        return nc.dram_tensor(name, shape, d, kind="ExternalInput")

    xT_d = din("xT", [IN_DIM, n_own])
    idxA_d = din("idxA", [128, TA * 8], dt.int16)
    idxB_d = din("idxB", [128, max(TB * 8, 16)], dt.int16)
    relA_d = din("relA", [128, TA])
    relB_d = din("relB", [128, max(TB, 1)])
    den_d = din("denInv", [O, segp_pad])
    iota_d = din("iota", [128, 128])
    f2cW_d = din("f2cW", [IN_DIM, O])
    f2cb_d = din("f2cb", [O, 1])
    rgcnW_d = din("rgcnW", [O, L * NUM_REL * O])
    rootW_d = din("rootW", [O, L * O])
    biasT_d = din("biasT", [O, L])
    gamma_d = din("gamma", [128, O])
    beta_d = din("beta", [128, O])
    out_d = nc.dram_tensor("out", [n_own, O], f32, kind="ExternalOutput")

    AluOp = mybir.AluOpType
    Act = mybir.ActivationFunctionType

    with tile.TileContext(nc) as tc:
        with (
            tc.tile_pool(name="persist", bufs=1) as pp,
            tc.tile_pool(name="msgp", bufs=10) as msgp,
            tc.tile_pool(name="selp", bufs=12) as selp,
            tc.tile_pool(name="rowp", bufs=6) as rowp,
            tc.tile_pool(name="lnp", bufs=8) as lnp,
            tc.tile_pool(name="strp", bufs=3) as strp,
            tc.tile_pool(name="psw", bufs=4, space="PSUM") as psw,
            tc.tile_pool(name="pspost", bufs=2, space="PSUM") as pspost,
            tc.tile_pool(name="pstr", bufs=2, space="PSUM") as pstr,
            tc.tile_pool(name="dram", bufs=1, space="DRAM") as dr,
        ):
            # ---- persistent SBUF tensors ----
            def persist(name, shape, d=f32):
                return pp.tile(shape, d, tag=name, name=name)

            idxA = persist("idxA", [128, TA * 8], dt.int16)
            idxB = persist("idxB", [128, max(TB * 8, 16)], dt.int16)
            relA = persist("relA", [128, TA])
            relB = persist("relB", [128, max(TB, 1)])
            iota = persist("iota", [128, 128])
            ident = persist("ident", [128, 128])
            f2cW = persist("f2cW", [IN_DIM, O])
            f2cb = persist("f2cb", [O, 1])
            rgcnW = persist("rgcnW", [O, L * NUM_REL * O])
            rootW = persist("rootW", [O, L * O])
            biasT = persist("biasT", [O, L])
            gamma = persist("gamma", [128, O])
            beta = persist("beta", [128, O])
            S_T = persist("S_T", [O, segp_pad])
            hT = persist("hT", [O, n_own])

            for sb_t, d_t in ((idxA, idxA_d), (idxB, idxB_d),
                              (relA, relA_d), (relB, relB_d),
                              (iota, iota_d), (f2cW, f2cW_d), (f2cb, f2cb_d),
                              (rgcnW, rgcnW_d), (rootW, rootW_d),
                              (biasT, biasT_d), (gamma, gamma_d),
                              (beta, beta_d)):
                nc.sync.dma_start(sb_t[:], d_t[:])
            make_identity(nc, ident[:])
            epscol = persist("epscol", [128, 1])
            nc.vector.memset(epscol[:], LN_EPS)
            nc.vector.memset(S_T[:], 0.0)

            # DRAM internals: per-layer bounce + gathered tables
            bounce = [dr.tile([n_own, O], f32, tag=f"bounce{l}", name=f"bounce{l}")
                      for l in range(L)]
            table = [dr.tile([N, O], f32, tag=f"table{l}", name=f"table{l}")
                     for l in range(L)]

            def chunks(total, step):
                return [(i, min(step, total - i)) for i in range(0, total, step)]

            # ---- layer 0 node features: h0T = f2cW.T @ xT (+bias) ----
            for (o, n) in chunks(n_own, 512):
                xTc = strp.tile([IN_DIM, 512], f32, tag="xTc", name="xTc")
                nc.sync.dma_start(xTc[:, :n], xT_d[:, o:o + n])
                ps = pspost.tile([O, 512], f32, tag="pspost", name="ps")
                nc.tensor.matmul(ps[:, :n], f2cW[:], xTc[:, :n],
                                 start=True, stop=True)
                nc.scalar.activation(hT[:, o:o + n], ps[:, :n], Act.Identity,
                                     bias=f2cb[:])

            def build_table(l):
                """transpose hT columns into row chunks, DMA to bounce, AllGather."""
                for (o, n) in chunks(n_own, 128):
                    ps = pstr.tile([128, O], f32, tag="pstr_rows", name="ps")
                    nc.tensor.matmul(ps[:n, :], hT[:, o:o + n], ident[:O, :O],
                                     start=True, stop=True)
                    rows = rowp.tile([128, O], f32, tag="rows", name="rows")
                    nc.scalar.activation(rows[:n, :], ps[:n, :], Act.Copy)
                    nc.sync.dma_start(bounce[l][o:o + n, :], rows[:n, :])
                if DBG_NO_COLLECTIVE:
                    nc.sync.dma_start(table[l][0:n_own, :], bounce[l][:])
                else:
                    nc.gpsimd.collective_compute(
                        "AllGather", AluOp.bypass,
                        replica_groups=[list(range(NCORES))],
                        ins=[bounce[l][:].opt()],
                        outs=[table[l][:].opt()],
                    )

            for l in range(L):
                build_table(l)

                # ---- segment phase: S_T[f, seg] = sum_e msg[e, f] ----
                for phase, (T_p, tiles_p, idx_p, rel_p, tbl_lo, tbl_n) in enumerate((
                    (TA, tilesA, idxA, relA, 0, rowsA),
                    (TB, tilesB, idxB, relB, HALF, max(N - HALF, 0)),
                )):
                    if T_p == 0:
                        continue
                    nbatch = _ceil(T_p, BATCH_TILES)
                    msgs = []
                    for b in range(nbatch):
                        t0 = b * BATCH_TILES
                        bt = min(BATCH_TILES, T_p - t0)
                        mbuf = msgp.tile([128, BATCH_TILES, O], f32, tag="msg",
                                         name="mbuf")
                        nc.gpsimd.dma_gather(
                            mbuf[:, :bt, :],
                            table[l][tbl_lo:tbl_lo + tbl_n, :],
                            idx_p[:, t0 * 8: t0 * 8 + bt * 8],
                            bt * 128, bt * 128, O,
                        )
                        msgs.append((t0, bt, mbuf))

                    ti = 0
                    for w in range(nwin):
                        nt = int(tiles_p[w])
                        if nt == 0:
                            continue
                        ps = psw.tile([O, WINSEG], f32, tag="psw", name="ps")
                        for k in range(nt):
                            t = ti + k
                            t0, bt, mbuf = msgs[t // BATCH_TILES]
                            sel = selp.tile([128, 128], f32, tag="sel",
                                            name="sel")
                            nc.vector.tensor_scalar(
                                out=sel[:], in0=iota[:],
                                scalar1=rel_p[:, t:t + 1], scalar2=None,
                                op0=AluOp.is_equal,
                            )
                            nc.tensor.matmul(
                                ps[:], mbuf[:, t - t0, :], sel[:],
                                start=(k == 0), stop=(k == nt - 1),
                            )
                        sl = S_T[:, w * WINSEG:(w + 1) * WINSEG]
                        if phase == 0:
                            # ACT copy: keeps DVE free for sel builds
                            nc.scalar.activation(sl, ps[:], Act.Copy)
                        else:
                            nc.vector.tensor_tensor(out=sl, in0=sl, in1=ps[:],
                                                    op=AluOp.add)
                        ti += nt
                    if phase == 0:
                        for w in range(nwin):
                            if int(tiles_p[w]) == 0:
                                nc.vector.memset(
                                    S_T[:, w * WINSEG:(w + 1) * WINSEG], 0.0)

                # ---- mean scaling (denominators streamed from DRAM) ----
                for (o, n) in chunks(segp_pad, 3 * 512):
                    denc = strp.tile([O, 3 * 512], f32, tag="denc", name="denc")
                    nc.sync.dma_start(denc[:, :n], den_d[:, o:o + n])
                    nc.vector.tensor_tensor(
                        out=S_T[:, o:o + n], in0=S_T[:, o:o + n],
                        in1=denc[:, :n], op=AluOp.mult)

                # ---- per-relation transform + root + bias + relu ----
                S_nr = S_T[:].rearrange("p (n r) -> p n r", r=NUM_REL)
                for (o, n) in chunks(n_own, 512):
                    ps = pspost.tile([O, 512], f32, tag="pspost", name="ps")
                    for r in range(NUM_REL):
                        ci = (l * NUM_REL + r) * O
                        nc.tensor.matmul(
                            ps[:, :n],
                            rgcnW[:, ci:ci + O],
                            S_nr[:, o:o + n, r],
                            start=(r == 0), stop=False,
                        )
                    nc.tensor.matmul(
                        ps[:, :n], rootW[:, l * O:(l + 1) * O], hT[:, o:o + n],
                        start=False, stop=True,
                    )
                    outTc = strp.tile([O, 512], f32, tag="outTc", name="outTc")
                    nc.scalar.activation(outTc[:, :n], ps[:, :n], Act.Relu,
                                         bias=biasT[:, l:l + 1])

                    # ---- transpose to rows + LayerNorm (128-node subchunks) --
                    for (o2, n2) in chunks(n, 128):
                        ps2 = pstr.tile([128, O], f32, tag="pstr_rows",
                                        name="ps2")
                        nc.tensor.matmul(ps2[:n2, :], outTc[:, o2:o2 + n2],
                                         ident[:O, :O], start=True, stop=True)
                        rows = rowp.tile([128, O], f32, tag="rows", name="rows")
                        musum = lnp.tile([128, 1], f32, tag="musum",
                                         name="musum")
                        nc.scalar.activation(rows[:n2, :], ps2[:n2, :], Act.Copy,
                                             accum_out=musum[:n2, :])
                        mu = lnp.tile([128, 1], f32, tag="mu", name="mu")
                        nc.vector.tensor_scalar(out=mu[:n2], in0=musum[:n2],
                                                scalar1=1.0 / O, scalar2=None,
                                                op0=AluOp.mult)
                        xc = lnp.tile([128, O], f32, tag="xc", name="xc")
                        nc.vector.tensor_scalar(out=xc[:n2, :], in0=rows[:n2, :],
                                                scalar1=mu[:n2], scalar2=None,
                                                op0=AluOp.subtract)
                        sq = lnp.tile([128, O], f32, tag="sq", name="sq")
                        varsum = lnp.tile([128, 1], f32, tag="varsum",
                                          name="varsum")
                        nc.scalar.activation(sq[:n2, :], xc[:n2, :], Act.Square,
                                             accum_out=varsum[:n2, :])
                        std = lnp.tile([128, 1], f32, tag="std", name="std")
                        nc.scalar.activation(std[:n2], varsum[:n2], Act.Sqrt,
                                             scale=1.0 / O, bias=epscol[:n2])
                        rstd = lnp.tile([128, 1], f32, tag="rstd", name="rstd")
                        nc.vector.reciprocal(rstd[:n2], std[:n2])
                        hrow = rowp.tile([128, O], f32, tag="hrow", name="hrow")
                        nc.vector.scalar_tensor_tensor(
                            out=hrow[:n2, :], in0=xc[:n2, :], scalar=rstd[:n2],
                            in1=gamma[:n2, :], op0=AluOp.mult, op1=AluOp.mult,
                        )
                        nc.vector.tensor_tensor(out=hrow[:n2, :],
                                                in0=hrow[:n2, :],
                                                in1=beta[:n2, :], op=AluOp.add)
                        go = o + o2
                        if l == L - 1:
                            nc.sync.dma_start(out_d[go:go + n2, :], hrow[:n2, :])
                        else:
                            psb = pstr.tile([O, 128], f32, tag="pstr_rows",
                                            name="psb")
                            nc.tensor.matmul(psb[:, :n2], hrow[:n2, :],
                                             ident[:n2, :n2],
                                             start=True, stop=True)
                            nc.scalar.activation(hT[:, go:go + n2], psb[:, :n2],
                                                 Act.Copy)

    nc.compile()
    return nc


def _make_in_maps(inputs, meta, cores, denom_inv):
    x = np.asarray(inputs["x"], dtype=np.float32)
    feat2c_W = inputs["feat2c_W"]
    feat2c_b = inputs["feat2c_b"]
    rgcn_W = inputs["rgcn_W"]
    rgcn_root = inputs["rgcn_root"]
    rgcn_bias = inputs["rgcn_bias"]
    ln_gamma = inputs["ln_gamma"]
    ln_beta = inputs["ln_beta"]
    N = x.shape[0]
    n_own = N // NCORES
    O = 64
    L = rgcn_W.shape[0]

    TB = meta["TB"]
    nwin = meta["nwin"]
    segp_pad = meta["segp_pad"]

    # shared (replicated) tensors
    iota = np.broadcast_to(np.arange(128, dtype=np.float32), (128, 128)).copy()
    f2cW = np.asarray(feat2c_W, dtype=np.float32)
    f2cb = np.asarray(feat2c_b, dtype=np.float32).reshape(O, 1)
    rgcnW = np.asarray(rgcn_W, dtype=np.float32).transpose(2, 0, 1, 3).reshape(
        O, L * NUM_REL * O)      # [d, (l r) o]
    rootW = np.asarray(rgcn_root, dtype=np.float32).transpose(1, 0, 2).reshape(
        O, L * O)
    biasT = np.asarray(rgcn_bias, dtype=np.float32).T.copy()   # [O, L]
    gamma = np.broadcast_to(np.asarray(ln_gamma, np.float32), (128, O)).copy()
    beta = np.broadcast_to(np.asarray(ln_beta, np.float32), (128, O)).copy()

    in_maps = []
    for c in range(NCORES):
        idxA, idxB, relA, relB = cores[c]
        den_c = denom_inv[c * n_own * NUM_REL:(c + 1) * n_own * NUM_REL]
        den64 = np.ones((O, segp_pad), dtype=np.float32)
        den64[:, :den_c.size] = den_c
        in_maps.append({
            "xT": x[c * n_own:(c + 1) * n_own, :].T.copy(),
            "idxA": _pack_idx(idxA),
            "idxB": _pack_idx(idxB) if TB > 0 else np.zeros((128, 16), np.int16),
            "relA": _pack_rel(relA),
            "relB": _pack_rel(relB) if TB > 0 else np.full((128, 1), -1.0, np.float32),
            "denInv": den64,
            "iota": iota,
            "f2cW": f2cW, "f2cb": f2cb, "rgcnW": rgcnW, "rootW": rootW,
            "biasT": biasT, "gamma": gamma, "beta": beta,
        })

    return in_maps


def _run(inputs, meta, cores, denom_inv, profile=False):
    import time

    from concourse.bass_utils import run_bass_kernel_spmd

    nc = _build_program(meta)
    in_maps = _make_in_maps(inputs, meta, cores, denom_inv)
    res = run_bass_kernel_spmd(nc, in_maps, core_ids=list(range(NCORES)))
    if profile:
        # no NTFF hook in this container: report min warm wall-clock
        # (includes tunnel transfer; upper bound on device time)
        best = None
        for _ in range(3):
            t0 = time.time()
            res = run_bass_kernel_spmd(nc, in_maps, core_ids=list(range(NCORES)))
            dt = time.time() - t0
            best = dt if best is None else min(best, dt)
        res.exec_time_ns = int(best * 1e9)
    out = np.concatenate([res.results[c]["out"] for c in range(NCORES)], axis=0)
    return out, res


def kernel(x, edge_index, edge_type, feat2c_W, feat2c_b, rgcn_W, rgcn_root,
           rgcn_bias, ln_gamma, ln_beta):
    inputs = dict(x=x, edge_index=edge_index, edge_type=edge_type,
                  feat2c_W=feat2c_W, feat2c_b=feat2c_b, rgcn_W=rgcn_W,
                  rgcn_root=rgcn_root, rgcn_bias=rgcn_bias,
                  ln_gamma=ln_gamma, ln_beta=ln_beta)
    meta, cores, denom_inv = _preprocess(
        np.asarray(x), np.asarray(edge_index), np.asarray(edge_type))
    out, _ = _run(inputs, meta, cores, denom_inv, profile=False)
    return out


if __name__ == "__main__":
    pass



# revision 4
# speedup vs baseline: 2.9448x; 2.9448x over previous
"""Trainium2 Bass kernel for DynamicGNN (3-layer RGCN-style message passing).

Strategy: shard destination nodes (and their incoming edges) across the 8
NeuronCores. Each core owns N/8 nodes = 3*N/8 (node,relation) segments.
Messages are gathered per-edge from a replicated f16 node-feature table in
DRAM via dma_gather, segment-reduced with selection-matrix matmuls on the
TensorEngine (mean denominators folded into the selection weights), then
transformed per-relation and layer-normed. Node tables are rebuilt each
layer with an AllGather collective into Shared scratchpad.

Host->device traffic is kept minimal: per core only xT [5,n_own] f32, a
compact int16 gather-index array (replicated to the 128-partition layout
on-device), an int8 (rel, count) slot-metadata array, and one packed f32
constants array. Output is f16 (cast to f32 on host).
"""
import math
import sys

import numpy as np

sys.path.insert(0, "/opt/trn_rl_repo")

NCORES = 8
HALF = 32768          # int16 index limit for dma_gather -> split table in 2
WINSEG = 128          # segments per psum window
BATCH_TILES = 8       # 128-edge tiles per dma_gather call (desc-ring limit ~1024)
LN_EPS = 1e-5
NUM_REL = 3

O = 64
L = 3
IN_DIM = 5

# const array column layout ([64, CC] f32)
C_F2CW = 0            # rows 0:5, cols 0:64
C_RGCNW = 64          # rows 0:64, 576 cols ((l*3+r)*64)
C_ROOTW = 640         # rows 0:64, 192 cols
C_BIAST = 832         # rows 0:64, 3 cols
C_F2CB = 835          # rows 0:64, 1 col
C_GAMMA = 836         # row 0, 64 cols
C_BETA = 900          # row 0, 64 cols
CC = 964


def _ceil(a, b):
    return (a + b - 1) // b


def _preprocess(x, edge_index, edge_type):
    """Host-side: shard edges by dst owner, sort by segment, build windows,
    per-phase slot streams (gather idx + per-slot rel/count metadata)."""
    N = x.shape[0]
    E = edge_index.shape[1]
    n_own = N // NCORES
    seg_per_core = n_own * NUM_REL
    nwin = _ceil(seg_per_core, WINSEG)

    src = edge_index[0].astype(np.int64)
    dst = edge_index[1].astype(np.int64)
    et = edge_type.astype(np.int64)

    owner = dst // n_own
    cntA = np.zeros((NCORES, nwin), dtype=np.int64)
    cntB = np.zeros((NCORES, nwin), dtype=np.int64)
    per_core = []
    for c in range(NCORES):
        m = owner == c
        s_c = src[m]
        seg_c = (dst[m] - c * n_own) * NUM_REL + et[m]
        order = np.argsort(seg_c, kind="stable")
        s_c = s_c[order]
        seg_c = seg_c[order]
        w_c = seg_c // WINSEG
        isA = s_c < HALF
        cntA[c] = np.bincount(w_c[isA], minlength=nwin)
        cntB[c] = np.bincount(w_c[~isA], minlength=nwin)
        # per-(local segment) counts for mean denominators
        segcnt = np.bincount(seg_c, minlength=seg_per_core)
        assert segcnt.max() <= 127, "int8 count overflow"
        per_core.append((s_c, seg_c, w_c, isA, segcnt))

    # compile-time tile structure: tiles per (phase, window) = max over cores
    tilesA = _ceil(np.maximum(cntA.max(axis=0), 0), 128)   # [nwin]
    tilesB = _ceil(np.maximum(cntB.max(axis=0), 0), 128)
    TA, TB = int(tilesA.sum()), int(tilesB.sum())
    slotsA, slotsB = TA * 128, TB * 128
    tbaseA = np.concatenate([[0], np.cumsum(tilesA)[:-1]])
    tbaseB = np.concatenate([[0], np.cumsum(tilesB)[:-1]])

    cores = []
    for c in range(NCORES):
        s_c, seg_c, w_c, isA, segcnt = per_core[c]
        idx_s = np.zeros(slotsA + slotsB, dtype=np.int16)
        rel_s = np.full(slotsA + slotsB, -1, dtype=np.int8)
        cnt_s = np.ones(slotsA + slotsB, dtype=np.int8)
        for (mask, tbase, soff, off) in (
            (isA, tbaseA, 0, 0),
            (~isA, tbaseB, slotsA, HALF),
        ):
            s_p = s_c[mask] - off
            seg_p = seg_c[mask]
            w_p = w_c[mask]
            gc = np.bincount(w_p, minlength=nwin)
            starts = np.concatenate([[0], np.cumsum(gc)[:-1]])
            pos = np.arange(len(w_p)) - starts[w_p]
            slot = soff + tbase[w_p] * 128 + pos
            idx_s[slot] = s_p.astype(np.int16)
            rel_s[slot] = (seg_p - w_p * WINSEG).astype(np.int8)
            cnt_s[slot] = segcnt[seg_p].astype(np.int8)
        # idx: [S] -> [16, S/16]; rel/cnt: [S] -> [128, S/128]
        idx16 = idx_s.reshape(-1, 16).T.copy()
        relp = rel_s.reshape(-1, 128).T
        cntp = cnt_s.reshape(-1, 128).T
        relcnt = np.concatenate([relp, cntp], axis=1).copy()
        cores.append((idx16, relcnt))

    segp_pad = _ceil(nwin * WINSEG, 3 * 512) * (3 * 512)
    meta = dict(
        N=N, E=E, n_own=n_own, seg_per_core=seg_per_core, nwin=nwin,
        tilesA=tilesA, tilesB=tilesB, TA=TA, TB=TB, segp_pad=segp_pad,
    )
    return meta, cores, None


def _build_program(meta):
    import concourse.bacc as bacc
    import concourse.bass as bass
    import concourse.mybir as mybir
    import concourse.tile as tile
    from concourse.masks import make_identity

    dt = mybir.dt
    f32 = dt.float32
    f16 = dt.float16
    N = meta["N"]
    n_own = meta["n_own"]
    nwin = meta["nwin"]
    tilesA, tilesB = meta["tilesA"], meta["tilesB"]
    TA, TB = meta["TA"], meta["TB"]
    T = TA + TB
    segp_pad = meta["segp_pad"]
    rowsA = HALF if N > HALF else N     # rows in table half A

    nc = bacc.Bacc("TRN2", target_bir_lowering=False, debug=False,
                   enable_asserts=False, num_devices=NCORES)

    xT_d = nc.dram_tensor("xT", [IN_DIM, n_own], f32, kind="ExternalInput")
    idx16_d = nc.dram_tensor("idx16", [16, T * 8], dt.int16,
                             kind="ExternalInput")
    relcnt_d = nc.dram_tensor("relcnt", [128, 2 * T], dt.int8,
                              kind="ExternalInput")
    const_d = nc.dram_tensor("konst", [O, CC], f32, kind="ExternalInput")
    out_d = nc.dram_tensor("out", [n_own, O], f16, kind="ExternalOutput")

    AluOp = mybir.AluOpType
    Act = mybir.ActivationFunctionType

    with tile.TileContext(nc) as tc:
        with (
            tc.tile_pool(name="persist", bufs=1) as pp,
            tc.tile_pool(name="msgpA", bufs=5) as msgpA,
            tc.tile_pool(name="msgpB", bufs=5) as msgpB,
            tc.tile_pool(name="selp", bufs=8) as selp,
            tc.tile_pool(name="rowp", bufs=6) as rowp,
            tc.tile_pool(name="lnp", bufs=8) as lnp,
            tc.tile_pool(name="strp", bufs=3) as strp,
            tc.tile_pool(name="psw", bufs=4, space="PSUM") as psw,
            tc.tile_pool(name="pspost", bufs=2, space="PSUM") as pspost,
            tc.tile_pool(name="pstr", bufs=2, space="PSUM") as pstr,
            tc.tile_pool(name="dram", bufs=1, space="DRAM") as dr,
        ):
            def persist(name, shape, d=f32):
                return pp.tile(shape, d, tag=name, name=name)

            idx_sb = persist("idx_sb", [128, T * 8], dt.int16)
            relcnt = persist("relcnt", [128, 2 * T], dt.int8)
            relf = persist("relf", [128, T])
            denvf = persist("denvf", [128, T])
            iota = persist("iota", [128, 128])
            ident = persist("ident", [128, 128])
            konst = persist("konst", [O, CC])
            gammaB = persist("gammaB", [128, O])
            betaB = persist("betaB", [128, O])
            onesrow = persist("onesrow", [1, 128])
            S_T = persist("S_T", [O, segp_pad])
            hT = persist("hT", [O, n_own])
            epscol = persist("epscol", [128, 1])

            nc.sync.dma_start(relcnt[:], relcnt_d[:])
            nc.sync.dma_start(konst[:], const_d[:])
            for g in range(8):
                nc.sync.dma_start(idx_sb[g * 16:(g + 1) * 16, :], idx16_d[:])

            make_identity(nc, ident[:])
            nc.gpsimd.iota(iota[:], [[1, 128]], channel_multiplier=0,
                           allow_small_or_imprecise_dtypes=True)
            nc.vector.memset(epscol[:], LN_EPS)
            nc.vector.memset(onesrow[:], 1.0)
            nc.vector.memset(S_T[:], 0.0)

            # decode slot metadata: rel (f32) and 1/count (f32)
            nc.vector.tensor_scalar(out=relf[:], in0=relcnt[:, :T],
                                    scalar1=0.0, scalar2=None, op0=AluOp.add)
            nc.vector.tensor_scalar(out=denvf[:], in0=relcnt[:, T:],
                                    scalar1=0.0, scalar2=None, op0=AluOp.add)
            nc.vector.reciprocal(denvf[:], denvf[:])

            # broadcast gamma/beta rows across 128 partitions via ones-matmul
            for (col, dst) in ((C_GAMMA, gammaB), (C_BETA, betaB)):
                psg = pstr.tile([128, O], f32, tag="pstr_rows", name="psg")
                nc.tensor.matmul(psg[:], onesrow[:],
                                 konst[0:1, col:col + O], start=True, stop=True)
                nc.scalar.activation(dst[:], psg[:], Act.Copy)

            # DRAM internals: per-layer bounce + gathered f16 tables
            bounce = [dr.tile([n_own, O], f32, tag=f"bounce{l}",
                              name=f"bounce{l}") for l in range(L)]
            table = [dr.tile([N, O], f32, tag=f"table{l}", name=f"table{l}",
                             addr_space="Shared") for l in range(L)]

            def chunks(total, step):
                return [(i, min(step, total - i)) for i in range(0, total, step)]

            # ---- layer 0 node features: h0T = f2cW.T @ xT (+bias) ----
            for (o, n) in chunks(n_own, 512):
                xTc = strp.tile([IN_DIM, 512], f32, tag="xTc", name="xTc")
                nc.sync.dma_start(xTc[:, :n], xT_d[:, o:o + n])
                ps = pspost.tile([O, 512], f32, tag="pspost", name="ps")
                nc.tensor.matmul(ps[:, :n], konst[0:IN_DIM, 0:O], xTc[:, :n],
                                 start=True, stop=True)
                nc.scalar.activation(hT[:, o:o + n], ps[:, :n], Act.Identity,
                                     bias=konst[:, C_F2CB:C_F2CB + 1])

            def build_table(l):
                """transpose hT columns into f16 row chunks, DMA to bounce,
                AllGather into Shared table."""
                for (o, n) in chunks(n_own, 128):
                    ps = pstr.tile([128, O], f32, tag="pstr_rows", name="ps")
                    nc.tensor.matmul(ps[:n, :], hT[:, o:o + n], ident[:O, :O],
                                     start=True, stop=True)
                    rows = rowp.tile([128, O], f32, tag="rows", name="rows")
                    nc.scalar.activation(rows[:n, :], ps[:n, :], Act.Copy)
                    nc.sync.dma_start(bounce[l][o:o + n, :], rows[:n, :])
                nc.gpsimd.collective_compute(
                    "AllGather", AluOp.bypass,
                    replica_groups=[list(range(NCORES))],
                    ins=[bounce[l][:].opt()],
                    outs=[table[l][:].opt()],
                )

            for l in range(L):
                build_table(l)

                # ---- gather per-edge messages (two table halves) ----
                msgsA, msgsB = [], []
                for (T_p, msgs, pool, tbl_lo, tbl_n, coff) in (
                    (TA, msgsA, msgpA, 0, rowsA, 0),
                    (TB, msgsB, msgpB, HALF, max(N - HALF, 0), TA * 8),
                ):
                    for b in range(_ceil(T_p, BATCH_TILES)):
                        t0 = b * BATCH_TILES
                        bt = min(BATCH_TILES, T_p - t0)
                        mbuf = pool.tile([128, BATCH_TILES, O], f32,
                                         tag="msg", name="mbuf")
                        nc.gpsimd.dma_gather(
                            mbuf[:, :bt, :],
                            table[l][tbl_lo:tbl_lo + tbl_n, :],
                            idx_sb[:, coff + t0 * 8: coff + (t0 + bt) * 8],
                            bt * 128, bt * 128, O,
                        )
                        msgs.append((t0, mbuf))

                # ---- segment sums: S_T[f, seg] = sum_e denv_e * msg[e, f] --
                tiA = tiB = 0
                for w in range(nwin):
                    ntA, ntB = int(tilesA[w]), int(tilesB[w])
                    nt = ntA + ntB
                    if nt == 0:
                        continue   # stays zero from the initial memset
                    ps = psw.tile([O, WINSEG], f32, tag="psw", name="ps")
                    k = 0
                    for (ti, ntp, msgs, toff) in ((tiA, ntA, msgsA, 0),
                                                  (tiB, ntB, msgsB, TA)):
                        for j in range(ntp):
                            t = ti + j
                            t0, mbuf = msgs[t // BATCH_TILES]
                            tc_col = toff + t
                            sel = selp.tile([128, 128], f32, tag="sel",
                                            name="sel")
                            nc.vector.tensor_scalar(
                                out=sel[:], in0=iota[:],
                                scalar1=relf[:, tc_col:tc_col + 1],
                                scalar2=denvf[:, tc_col:tc_col + 1],
                                op0=AluOp.is_equal, op1=AluOp.mult,
                            )
                            nc.tensor.matmul(
                                ps[:], mbuf[:, t - t0, :], sel[:],
                                start=(k == 0), stop=(k == nt - 1),
                            )
                            k += 1
                    tiA += ntA
                    tiB += ntB
                    nc.scalar.activation(
                        S_T[:, w * WINSEG:(w + 1) * WINSEG], ps[:], Act.Copy)

                # ---- per-relation transform + root + bias + relu ----
                S_nr = S_T[:].rearrange("p (n r) -> p n r", r=NUM_REL)
                for (o, n) in chunks(n_own, 512):
                    ps = pspost.tile([O, 512], f32, tag="pspost", name="ps")
                    for r in range(NUM_REL):
                        ci = C_RGCNW + (l * NUM_REL + r) * O
                        nc.tensor.matmul(
                            ps[:, :n], konst[:, ci:ci + O], S_nr[:, o:o + n, r],
                            start=(r == 0), stop=False,
                        )
                    ci = C_ROOTW + l * O
                    nc.tensor.matmul(
                        ps[:, :n], konst[:, ci:ci + O], hT[:, o:o + n],
                        start=False, stop=True,
                    )
                    outTc = strp.tile([O, 512], f32, tag="outTc", name="outTc")
                    nc.scalar.activation(outTc[:, :n], ps[:, :n], Act.Relu,
                                         bias=konst[:, C_BIAST + l:C_BIAST + l + 1])

                    # ---- transpose to rows + LayerNorm (128-node subchunks) --
                    for (o2, n2) in chunks(n, 128):
                        ps2 = pstr.tile([128, O], f32, tag="pstr_rows",
                                        name="ps2")
                        nc.tensor.matmul(ps2[:n2, :], outTc[:, o2:o2 + n2],
                                         ident[:O, :O], start=True, stop=True)
                        rows = rowp.tile([128, O], f32, tag="rows", name="rows")
                        musum = lnp.tile([128, 1], f32, tag="musum",
                                         name="musum")
                        nc.scalar.activation(rows[:n2, :], ps2[:n2, :], Act.Copy,
                                             accum_out=musum[:n2, :])
                        mu = lnp.tile([128, 1], f32, tag="mu", name="mu")
                        nc.vector.tensor_scalar(out=mu[:n2], in0=musum[:n2],
                                                scalar1=1.0 / O, scalar2=None,
                                                op0=AluOp.mult)
                        xc = lnp.tile([128, O], f32, tag="xc", name="xc")
                        nc.vector.tensor_scalar(out=xc[:n2, :], in0=rows[:n2, :],
                                                scalar1=mu[:n2], scalar2=None,
                                                op0=AluOp.subtract)
                        sq = lnp.tile([128, O], f32, tag="sq", name="sq")
                        varsum = lnp.tile([128, 1], f32, tag="varsum",
                                          name="varsum")
                        nc.scalar.activation(sq[:n2, :], xc[:n2, :], Act.Square,
                                             accum_out=varsum[:n2, :])
                        std = lnp.tile([128, 1], f32, tag="std", name="std")
                        nc.scalar.activation(std[:n2], varsum[:n2], Act.Sqrt,
                                             scale=1.0 / O, bias=epscol[:n2])
                        rstd = lnp.tile([128, 1], f32, tag="rstd", name="rstd")
                        nc.vector.reciprocal(rstd[:n2], std[:n2])
                        hrow = rowp.tile([128, O], f32, tag="hrow", name="hrow")
                        nc.vector.scalar_tensor_tensor(
                            out=hrow[:n2, :], in0=xc[:n2, :], scalar=rstd[:n2],
                            in1=gammaB[:n2, :], op0=AluOp.mult, op1=AluOp.mult,
                        )
                        go = o + o2
                        if l == L - 1:
                            out16 = rowp.tile([128, O], f16, tag="out16",
                                              name="out16")
                            nc.vector.tensor_tensor(out=out16[:n2, :],
                                                    in0=hrow[:n2, :],
                                                    in1=betaB[:n2, :],
                                                    op=AluOp.add)
                            nc.sync.dma_start(out_d[go:go + n2, :],
                                              out16[:n2, :])
                        else:
                            nc.vector.tensor_tensor(out=hrow[:n2, :],
                                                    in0=hrow[:n2, :],
                                                    in1=betaB[:n2, :],
                                                    op=AluOp.add)
                            psb = pstr.tile([O, 128], f32, tag="pstr_rows",
                                            name="psb")
                            nc.tensor.matmul(psb[:, :n2], hrow[:n2, :],
                                             ident[:n2, :n2],
                                             start=True, stop=True)
                            nc.scalar.activation(hT[:, go:go + n2], psb[:, :n2],
                                                 Act.Copy)

    nc.compile()
    return nc


def _make_in_maps(inputs, meta, cores, denom_inv=None):
    x = np.asarray(inputs["x"], dtype=np.float32)
    N = x.shape[0]
    n_own = N // NCORES

    konst = np.zeros((O, CC), dtype=np.float32)
    konst[:IN_DIM, 0:O] = np.asarray(inputs["feat2c_W"], np.float32)
    konst[:, C_RGCNW:C_RGCNW + L * NUM_REL * O] = (
        np.asarray(inputs["rgcn_W"], np.float32)
        .transpose(2, 0, 1, 3).reshape(O, L * NUM_REL * O))
    konst[:, C_ROOTW:C_ROOTW + L * O] = (
        np.asarray(inputs["rgcn_root"], np.float32)
        .transpose(1, 0, 2).reshape(O, L * O))
    konst[:, C_BIAST:C_BIAST + L] = np.asarray(
        inputs["rgcn_bias"], np.float32).T
    konst[:, C_F2CB] = np.asarray(inputs["feat2c_b"], np.float32)
    konst[0, C_GAMMA:C_GAMMA + O] = np.asarray(inputs["ln_gamma"], np.float32)
    konst[0, C_BETA:C_BETA + O] = np.asarray(inputs["ln_beta"], np.float32)

    in_maps = []
    for c in range(NCORES):
        idx16, relcnt = cores[c]
        in_maps.append({
            "xT": np.ascontiguousarray(x[c * n_own:(c + 1) * n_own, :].T),
            "idx16": idx16,
            "relcnt": relcnt,
            "konst": konst,
        })
    return in_maps


def _run(inputs, meta, cores, denom_inv=None, profile=False):
    import time

    from concourse.bass_utils import run_bass_kernel_spmd

    nc = _build_program(meta)
    in_maps = _make_in_maps(inputs, meta, cores)
    res = run_bass_kernel_spmd(nc, in_maps, core_ids=list(range(NCORES)))
    if profile:
        # no NTFF hook in this container: report min warm wall-clock
        # (includes tunnel transfer; upper bound on device time)
        best = None
        for _ in range(3):
            t0 = time.time()
            res = run_bass_kernel_spmd(nc, in_maps, core_ids=list(range(NCORES)))
            dt = time.time() - t0
            best = dt if best is None else min(best, dt)
        res.exec_time_ns = int(best * 1e9)
    out = np.concatenate(
        [res.results[c]["out"] for c in range(NCORES)], axis=0
    ).astype(np.float32)
    return out, res


def kernel(x, edge_index, edge_type, feat2c_W, feat2c_b, rgcn_W, rgcn_root,
           rgcn_bias, ln_gamma, ln_beta):
    inputs = dict(x=x, edge_index=edge_index, edge_type=edge_type,
                  feat2c_W=feat2c_W, feat2c_b=feat2c_b, rgcn_W=rgcn_W,
                  rgcn_root=rgcn_root, rgcn_bias=rgcn_bias,
                  ln_gamma=ln_gamma, ln_beta=ln_beta)
    meta, cores, _ = _preprocess(
        np.asarray(x), np.asarray(edge_index), np.asarray(edge_type))
    out, _ = _run(inputs, meta, cores, profile=False)
    return out


if __name__ == "__main__":
    pass
